# revision 11
# baseline (speedup 1.0000x reference)
"""BuNN hop layer on 8 Trainium2 NeuronCores.

Strategy (graph/data parallel per the sharding hint):
  - nodes sharded 8 ways by id; each core owns 12500 nodes (padded to 12544)
  - full node-state table (f32, 512B rows) replicated per core in HBM
  - propagation step: per-core dma_gather of in-edge source rows (512B
    indexed gathers run at full DMA rate), segment-sum via TensorE matmuls
    with on-the-fly one-hot indicator matrices (iota + is_equal on DVE),
    accumulated in PSUM per 128-dst window, scaled by 1/deg, then an
    AllGather refreshes the replicas
  - struct-enc MLP, bundle rotations, lin map, attention-combine and the
    residual are all node-local, done feature-major with small matmuls
Host side does pure index preprocessing: bucket edges by (dst-window,
src-chunk), pad buckets to 128-multiples, pack int16 gather indices.
"""

import sys

sys.path.insert(0, "/opt/trn_rl_repo")

import math
import numpy as np

import concourse.bass as bass
import concourse.mybir as mybir
from concourse import tile, bacc
from concourse.bass_utils import run_bass_kernel_spmd

F32 = mybir.dt.float32
I16 = mybir.dt.int16
I32 = mybir.dt.int32

N = 100000
C = 128
NB = 32
NCORES = 8
SHARD = 12500  # owned nodes per core
SLOTS = 12544  # padded shard slots (98 * 128)
WIN = SLOTS // 128  # 98 windows
TROWS = NCORES * SLOTS  # 100352 table rows
CHUNK = 32768
NCH = (TROWS + CHUNK - 1) // CHUNK  # 4 (last chunk 2048 rows)
SBW = 4  # windows per superblock (gather batching)
NSTEPS = 8
CKPT_AFTER = {1: 0, 2: 1, 4: 2, 8: 3}


# ---------------------------------------------------------------- host prep
def _preprocess(edge_index):
    src_g = np.asarray(edge_index[0], dtype=np.int64)
    dst_g = np.asarray(edge_index[1], dtype=np.int64)
    deg = (np.bincount(src_g, minlength=N) + 1).astype(np.float32)

    src_r = (src_g // SHARD) * SLOTS + (src_g % SHARD)
    dst_r = (dst_g // SHARD) * SLOTS + (dst_g % SHARD)
    core_of = dst_g // SHARD

    per_core = []
    counts = np.zeros((NCORES, WIN, NCH), np.int64)
    for c in range(NCORES):
        m = core_of == c
        s = src_r[m]
        d_local = dst_r[m] - c * SLOTS
        w = d_local >> 7
        slot = d_local & 127
        k = s >> 15
        sb = w // SBW
        order = np.lexsort((slot, w, k, sb))  # sb-major, then chunk, then window
        per_core.append((s[order], w[order], slot[order], k[order], sb[order]))
        np.add.at(counts[c], (w, k), 1)

    cols = np.ceil(counts.max(axis=0) / 128).astype(np.int64)  # [WIN, NCH]

    # global column layout: for each sb, for each chunk, for each window in sb
    n_sb = (WIN + SBW - 1) // SBW
    col_off = np.zeros((WIN, NCH), np.int64)
    sched = []  # per sb: {windows, gathers, col0, ncols, wcols, last_gc}
    off = 0
    for sbi in range(n_sb):
        ws = list(range(sbi * SBW, min((sbi + 1) * SBW, WIN)))
        sb_col0 = off
        gathers = []
        for k in range(NCH):
            g0 = off
            for w in ws:
                col_off[w, k] = off
                off += int(cols[w, k])
            if off > g0:
                gathers.append((k, g0, off - g0))
        wcols = {}
        last_gc = {}
        for w in ws:
            runs = [
                (k, int(col_off[w, k]), int(cols[w, k]))
                for k in range(NCH)
                if cols[w, k] > 0
            ]
            assert runs, f"window {w} has no edge columns"
            wcols[w] = runs
            last_gc[w] = runs[-1][1] + runs[-1][2] - 1
        sched.append(
            {
                "windows": ws,
                "gathers": gathers,
                "col0": sb_col0,
                "ncols": off - sb_col0,
                "wcols": wcols,
                "last_gc": last_gc,
            }
        )
    totcols = off

    idx_packs, dst_packs = [], []
    for c in range(NCORES):
        s, w, slot, k, sb = per_core[c]
        idx_flat = np.zeros(totcols * 128, np.int16)
        dst_flat = np.full(totcols * 128, -1.0, np.float32)
        # edges sorted by (sb, k, w); fill bucket by bucket
        ptr = 0
        for sbi in range(n_sb):
            for ki in range(NCH):
                for wi in sched[sbi]["windows"]:
                    n = counts[c, wi, ki]
                    o = col_off[wi, ki] * 128
                    idx_flat[o : o + n] = (s[ptr : ptr + n] - ki * CHUNK).astype(
                        np.int16
                    )
                    dst_flat[o : o + n] = slot[ptr : ptr + n]
                    ptr += n
        assert ptr == len(s)
        idx_packs.append(
            np.tile(np.ascontiguousarray(idx_flat.reshape(-1, 16).T), (8, 1))
        )
        dst_packs.append(np.ascontiguousarray(dst_flat.reshape(-1, 128).T))

    return {
        "deg": deg,
        "cols": cols,
        "col_off": col_off,
        "sched": sched,
        "totcols": totcols,
        "idx_packs": idx_packs,
        "dst_packs": dst_packs,
    }


# ---------------------------------------------------------------- builder
def _build(sched, totcols, debug_out=None):
    """debug_out: None -> full kernel; 'h0' -> stop after preproc;
    int s -> stop after prop step s (output that state, feature-major)."""
    nc = bacc.Bacc("TRN2", target_bir_lowering=False, debug=False, num_devices=NCORES)

    # ---------------- parameters (per-core shards / replicated smalls)
    xT = nc.declare_dram_parameter("xT", [128, SLOTS], F32, isOutput=False)
    idxs_d = nc.declare_dram_parameter(
        "idxs", [128, totcols * 8], I16, isOutput=False
    )
    dstloc_d = nc.declare_dram_parameter("dstloc", [128, totcols], F32, isOutput=False)
    degc_d = nc.declare_dram_parameter("degc", [128, WIN], F32, isOutput=False)
    encw1_d = nc.declare_dram_parameter("encw1", [128, 64], F32, isOutput=False)
    encb1_d = nc.declare_dram_parameter("encb1", [64, 1], F32, isOutput=False)
    encw2_d = nc.declare_dram_parameter("encw2", [64, 32], F32, isOutput=False)
    encb2_d = nc.declare_dram_parameter("encb2", [32, 1], F32, isOutput=False)
    linw_d = nc.declare_dram_parameter("linw", [128, 128], F32, isOutput=False)
    linb_d = nc.declare_dram_parameter("linb", [128, 1], F32, isOutput=False)
    attT_d = nc.declare_dram_parameter("attT", [32, 4], F32, isOutput=False)
    iota_d = nc.declare_dram_parameter("iotar", [128, 128], F32, isOutput=False)
    E4_d = nc.declare_dram_parameter("E4", [32, 128], F32, isOutput=False)
    E4s_d = nc.declare_dram_parameter("E4s", [32, 128], F32, isOutput=False)
    E4si_d = nc.declare_dram_parameter("E4si", [32, 128], F32, isOutput=False)
    P128_d = nc.declare_dram_parameter("P128", [128, 128], F32, isOutput=False)
    ID128_d = nc.declare_dram_parameter("ID128", [128, 128], F32, isOutput=False)
    out_d = nc.declare_dram_parameter("out", [128, SLOTS], F32, isOutput=True)

    # ---------------- internal DRAM
    own = [nc.dram_tensor(f"own{i}", [SLOTS, 128], F32) for i in range(2)]
    tbl = [
        nc.dram_tensor(f"tbl{i}", [TROWS, 128], F32, addr_space="Shared")
        for i in range(2)
    ]
    ck = [nc.dram_tensor(f"ck{i}", [SLOTS, 128], F32) for i in range(4)]
    cstore = nc.dram_tensor("cstore", [32, SLOTS], F32)
    sstore = nc.dram_tensor("sstore", [32, SLOTS], F32)

    RG = [list(range(NCORES))]
    PI = math.pi

    def own_nm(buf, w0, nw):
        # own[w*128+p, f] viewed as [p, w, f] for windows [w0, w0+nw)
        return buf.ap().rearrange("(s p) f -> p s f", p=128)[:, w0 : w0 + nw, :]

    with tile.TileContext(nc) as tc:
        with (
            tc.tile_pool(name="const", bufs=1) as constp,
            tc.tile_pool(name="resid", bufs=1) as residp,
        ):
            # resident constants
            iota_f = constp.tile([128, 128], F32)
            nc.sync.dma_start(out=iota_f[:], in_=iota_d[:])
            E4 = constp.tile([32, 128], F32)
            nc.sync.dma_start(out=E4[:], in_=E4_d[:])
            E4s = constp.tile([32, 128], F32)
            nc.sync.dma_start(out=E4s[:], in_=E4s_d[:])
            E4si = constp.tile([32, 128], F32)
            nc.sync.dma_start(out=E4si[:], in_=E4si_d[:])
            P128 = constp.tile([128, 128], F32)
            nc.sync.dma_start(out=P128[:], in_=P128_d[:])
            ID128 = constp.tile([128, 128], F32)
            nc.sync.dma_start(out=ID128[:], in_=ID128_d[:])
            linw = constp.tile([128, 128], F32)
            nc.sync.dma_start(out=linw[:], in_=linw_d[:])
            linb = constp.tile([128, 1], F32)
            nc.sync.dma_start(out=linb[:], in_=linb_d[:])
            encw1 = constp.tile([128, 64], F32)
            nc.sync.dma_start(out=encw1[:], in_=encw1_d[:])
            encb1 = constp.tile([64, 1], F32)
            nc.sync.dma_start(out=encb1[:], in_=encb1_d[:])
            encw2 = constp.tile([64, 32], F32)
            nc.sync.dma_start(out=encw2[:], in_=encw2_d[:])
            encb2 = constp.tile([32, 1], F32)
            nc.sync.dma_start(out=encb2[:], in_=encb2_d[:])

            # resident big data
            idxs = residp.tile([128, totcols * 8], I16)
            nc.sync.dma_start(out=idxs[:], in_=idxs_d[:])
            dstloc = residp.tile([128, totcols], F32)
            nc.sync.dma_start(out=dstloc[:], in_=dstloc_d[:])
            degc = residp.tile([128, WIN], F32)
            nc.sync.dma_start(out=degc[:], in_=degc_d[:])
            invdeg = residp.tile([128, WIN], F32)
            nc.vector.reciprocal(invdeg[:], degc[:])
            pio2 = residp.tile([32, 1], F32)
            nc.vector.memset(pio2[:], PI / 2)

            # ---------------- preprocessing: theta MLP, rotation, lin
            NT = SLOTS // 512  # 24.5 -> SLOTS=12544 = 24*512 + 256; use 512 tiles
            tiles = [(i * 512, 512) for i in range(SLOTS // 512)]
            if SLOTS % 512:
                tiles.append((SLOTS - SLOTS % 512, SLOTS % 512))
            with (
                tc.tile_pool(name="pre_sb", bufs=3) as pre,
                tc.tile_pool(name="pre_ps", bufs=1, space="PSUM") as pps,
                tc.tile_pool(name="pre_ps2", bufs=1, space="PSUM") as pps2,
                tc.tile_pool(name="pre_ps3", bufs=2, space="PSUM") as pps3,
                tc.tile_pool(name="tr_ps", bufs=1, space="PSUM") as trps,
            ):
                for t0, tw in tiles:
                    xt = pre.tile([128, 512], F32, tag="xt")
                    nc.sync.dma_start(out=xt[:, :tw], in_=xT[:, t0 : t0 + tw])
                    mid_ps = pps.tile([64, 512], F32, tag="mid")
                    nc.tensor.matmul(
                        mid_ps[:, :tw], encw1[:], xt[:, :tw], start=True, stop=True
                    )
                    mid = pre.tile([64, 512], F32, tag="mid_sb")
                    nc.scalar.activation(
                        mid[:, :tw],
                        mid_ps[:, :tw],
                        mybir.ActivationFunctionType.Gelu,
                        bias=encb1[:, 0:1],
                    )
                    th_ps = pps.tile([32, 512], F32, tag="th")
                    nc.tensor.matmul(
                        th_ps[:, :tw], encw2[:], mid[:, :tw], start=True, stop=True
                    )
                    th = pre.tile([32, 512], F32, tag="th_sb")
                    nc.scalar.activation(
                        th[:, :tw],
                        th_ps[:, :tw],
                        mybir.ActivationFunctionType.Tanh,
                        bias=encb2[:, 0:1],
                    )
                    # c = 1 - 2*sin^2(theta/2), s = sin(theta); theta = pi*th
                    uh = pre.tile([32, 512], F32, tag="uh")
                    nc.scalar.activation(
                        uh[:, :tw],
                        th[:, :tw],
                        mybir.ActivationFunctionType.Sin,
                        bias=0.0,
                        scale=PI / 2,
                    )
                    u2 = pre.tile([32, 512], F32, tag="u2")
                    nc.vector.tensor_tensor(
                        u2[:, :tw], uh[:, :tw], uh[:, :tw], mybir.AluOpType.mult
                    )
                    c_t = pre.tile([32, 512], F32, tag="c_t")
                    nc.vector.tensor_scalar(
                        c_t[:, :tw],
                        u2[:, :tw],
                        -2.0,
                        1.0,
                        mybir.AluOpType.mult,
                        mybir.AluOpType.add,
                    )
                    s_t = pre.tile([32, 512], F32, tag="s_t")
                    nc.scalar.activation(
                        s_t[:, :tw],
                        th[:, :tw],
                        mybir.ActivationFunctionType.Sin,
                        bias=0.0,
                        scale=PI,
                    )
                    nc.sync.dma_start(out=cstore[:, t0 : t0 + tw], in_=c_t[:, :tw])
                    nc.sync.dma_start(out=sstore[:, t0 : t0 + tw], in_=s_t[:, :tw])

                    cexp_ps = pps2.tile([128, 512], F32, tag="cexp")
                    nc.tensor.matmul(
                        cexp_ps[:, :tw], E4[:], c_t[:, :tw], start=True, stop=True
                    )
                    sexp_ps = pps2.tile([128, 512], F32, tag="sexp")
                    nc.tensor.matmul(
                        sexp_ps[:, :tw], E4s[:], s_t[:, :tw], start=True, stop=True
                    )
                    sexp = pre.tile([128, 512], F32, tag="sexp_sb")
                    nc.vector.tensor_copy(sexp[:, :tw], sexp_ps[:, :tw])
                    xsw_ps = pps2.tile([128, 512], F32, tag="xsw")
                    nc.tensor.matmul(
                        xsw_ps[:, :tw], P128[:], xt[:, :tw], start=True, stop=True
                    )
                    t1 = pre.tile([128, 512], F32, tag="t1")
                    nc.vector.tensor_tensor(
                        t1[:, :tw], cexp_ps[:, :tw], xt[:, :tw], mybir.AluOpType.mult
                    )
                    t2 = pre.tile([128, 512], F32, tag="t2")
                    nc.vector.tensor_tensor(
                        t2[:, :tw], xsw_ps[:, :tw], sexp[:, :tw], mybir.AluOpType.mult
                    )
                    rot = pre.tile([128, 512], F32, tag="rot")
                    nc.vector.tensor_tensor(
                        rot[:, :tw], t1[:, :tw], t2[:, :tw], mybir.AluOpType.add
                    )
                    h0_ps = pps3.tile([128, 512], F32, tag="h0")
                    nc.tensor.matmul(
                        h0_ps[:, :tw], linw[:], rot[:, :tw], start=True, stop=True
                    )
                    h0t = pre.tile([128, 512], F32, tag="h0t")
                    nc.vector.tensor_scalar(
                        h0t[:, :tw],
                        h0_ps[:, :tw],
                        linb[:, 0:1],
                        None,
                        mybir.AluOpType.add,
                    )
                    if debug_out == "h0":
                        nc.sync.dma_start(out=out_d[:, t0 : t0 + tw], in_=h0t[:, :tw])
                    # transpose to node-major, store into own[0]
                    for q in range(tw // 128):
                        tr = trps.tile([128, 128], F32, tag="tr")
                        nc.tensor.transpose(
                            tr[:], h0t[:, q * 128 : (q + 1) * 128], ID128[:]
                        )
                        nm = pre.tile([128, 128], F32, tag="nm")
                        nc.vector.tensor_copy(nm[:], tr[:])
                        w = (t0 + q * 128) // 128
                        nc.sync.dma_start(out=own_nm(own[0], w, 1)[:, 0, :], in_=nm[:])

            # h0 allgather
            nc.gpsimd.collective_compute(
                "AllGather",
                mybir.AluOpType.bypass,
                replica_groups=RG,
                ins=[own[0].ap().opt()],
                outs=[tbl[0].ap().opt()],
            )

            # ---------------- propagation steps
            nsteps = NSTEPS if debug_out in (None, "full") else (
                0 if debug_out == "h0" else int(debug_out)
            )
            with (
                tc.tile_pool(name="gath", bufs=2) as gath_pool,
                tc.tile_pool(name="ind", bufs=4) as ind_pool,
                tc.tile_pool(name="selfp", bufs=2) as self_pool,
                tc.tile_pool(name="drain", bufs=4) as drain_pool,
                tc.tile_pool(name="prop_ps", bufs=6, space="PSUM") as prop_ps,
            ):
                sbcols_max = max(sb["ncols"] for sb in sched)
                for step in range(1, nsteps + 1):
                    cur, nxt = (step + 1) % 2, step % 2
                    for sb in sched:
                        sbbuf = gath_pool.tile([128, sbcols_max, 128], F32, tag="g")
                        for k, g0, ncols in sb["gathers"]:
                            krows = min(CHUNK, TROWS - k * CHUNK)
                            nc.gpsimd.dma_gather(
                                out_ap=sbbuf[:, g0 - sb["col0"] : g0 - sb["col0"] + ncols, :],
                                in_ap=tbl[cur].ap()[k * CHUNK : k * CHUNK + krows, :],
                                idxs_ap=idxs[:, g0 * 8 : (g0 + ncols) * 8],
                                num_idxs=ncols * 128,
                                num_idxs_reg=ncols * 128,
                                elem_size=128,
                                single_packet=False,
                            )
                        ws = sb["windows"]
                        selft = self_pool.tile([128, SBW, 128], F32, tag="s")
                        nc.sync.dma_start(
                            out=selft[:, : len(ws), :], in_=own_nm(own[cur], ws[0], len(ws))
                        )
                        for wi, w in enumerate(ws):
                            psum = prop_ps.tile([128, 128], F32, tag="pp")
                            first = True
                            for (k, gc0, gcn) in sb["wcols"][w]:
                                for t in range(gcn):
                                    gc = gc0 + t
                                    ind = ind_pool.tile([128, 128], F32, tag="i")
                                    nc.vector.tensor_scalar(
                                        ind[:],
                                        iota_f[:],
                                        dstloc[:, gc : gc + 1],
                                        None,
                                        mybir.AluOpType.is_equal,
                                    )
                                    nc.tensor.matmul(
                                        psum[:],
                                        ind[:],
                                        sbbuf[:, gc - sb["col0"], :],
                                        start=first,
                                        stop=(gc == sb["last_gc"][w]),
                                    )
                                    first = False
                            tadd = drain_pool.tile([128, 128], F32, tag="a")
                            nc.vector.tensor_tensor(
                                tadd[:], psum[:], selft[:, wi, :], mybir.AluOpType.add
                            )
                            tout = drain_pool.tile([128, 128], F32, tag="o")
                            nc.vector.tensor_scalar(
                                tout[:],
                                tadd[:],
                                invdeg[:, w : w + 1],
                                None,
                                mybir.AluOpType.mult,
                            )
                            nc.sync.dma_start(
                                out=own_nm(own[nxt], w, 1)[:, 0, :], in_=tout[:]
                            )
                            if isinstance(debug_out, int) and step == nsteps:
                                nc.sync.dma_start(
                                    out=out_d.ap().rearrange(
                                        "p (s f) -> p s f", f=128
                                    )[:, w, :],
                                    in_=tout[:],
                                )
                            if step in CKPT_AFTER:
                                nc.sync.dma_start(
                                    out=own_nm(ck[CKPT_AFTER[step]], w, 1)[:, 0, :],
                                    in_=tout[:],
                                )
                    if step < nsteps or debug_out in (None, "full"):
                        nc.gpsimd.collective_compute(
                            "AllGather",
                            mybir.AluOpType.bypass,
                            replica_groups=RG,
                            ins=[own[nxt].ap().opt()],
                            outs=[tbl[nxt].ap().opt()],
                        )


            if debug_out in (None, "full"):
                # ---------------- postprocessing
                with (
                    tc.tile_pool(name="post_sb", bufs=3) as post,
                    tc.tile_pool(name="post_ps", bufs=1, space="PSUM") as pops,
                    tc.tile_pool(name="tr2_ps", bufs=4, space="PSUM") as trps2,
                ):
                    # attention softmax + expansion
                    attT = post.tile([32, 4], F32, tag="attT")
                    nc.sync.dma_start(out=attT[:], in_=attT_d[:])
                    ex = post.tile([32, 4], F32, tag="attex")
                    nc.scalar.activation(
                        ex[:], attT[:], mybir.ActivationFunctionType.Exp
                    )
                    ssum = post.tile([32, 1], F32, tag="attsum")
                    nc.vector.reduce_sum(ssum[:], ex[:], mybir.AxisListType.X)
                    rsum = post.tile([32, 1], F32, tag="attr")
                    nc.vector.reciprocal(rsum[:], ssum[:])
                    attw = post.tile([32, 4], F32, tag="attw")
                    nc.vector.tensor_scalar(
                        attw[:], ex[:], rsum[:, 0:1], None, mybir.AluOpType.mult
                    )
                    aw_ps = pops.tile([128, 4], F32, tag="awps")
                    nc.tensor.matmul(aw_ps[:], E4[:], attw[:], start=True, stop=True)
                    attwe = post.tile([128, 4], F32, tag="attwe")
                    nc.vector.tensor_copy(attwe[:], aw_ps[:])

                    for g0 in range(0, WIN, SBW):
                        nw = min(SBW, WIN - g0)
                        ckt = []
                        for a in range(4):
                            ckta = post.tile([128, SBW, 128], F32, tag=f"ck{a}")
                            ckt.append(ckta)
                        for a in range(4):
                            nc.sync.dma_start(
                                out=ckt[a][:, :nw, :], in_=own_nm(ck[a], g0, nw)
                            )
                        c_t = post.tile([32, SBW * 128], F32, tag="pc")
                        nc.sync.dma_start(
                            out=c_t[:, : nw * 128],
                            in_=cstore[:, g0 * 128 : (g0 + nw) * 128],
                        )
                        s_t = post.tile([32, SBW * 128], F32, tag="ps")
                        nc.sync.dma_start(
                            out=s_t[:, : nw * 128],
                            in_=sstore[:, g0 * 128 : (g0 + nw) * 128],
                        )
                        xt = post.tile([128, SBW * 128], F32, tag="px")
                        nc.sync.dma_start(
                            out=xt[:, : nw * 128],
                            in_=xT[:, g0 * 128 : (g0 + nw) * 128],
                        )
                        for wi in range(nw):
                            w = g0 + wi
                            # combine checkpoints with attention weights
                            acc = None
                            for a in range(4):
                                trp = trps2.tile([128, 128], F32, tag="tr2")
                                nc.tensor.transpose(trp[:], ckt[a][:, wi, :], ID128[:])
                                nacc = post.tile([128, 128], F32, tag=f"acc{a%2}")
                                if a == 0:
                                    nc.vector.tensor_scalar(
                                        nacc[:],
                                        trp[:],
                                        attwe[:, 0:1],
                                        None,
                                        mybir.AluOpType.mult,
                                    )
                                else:
                                    nc.vector.scalar_tensor_tensor(
                                        nacc[:],
                                        trp[:],
                                        attwe[:, a : a + 1],
                                        acc[:],
                                        mybir.AluOpType.mult,
                                        mybir.AluOpType.add,
                                    )
                                acc = nacc
                            # inverse rotation + residual
                            cw = c_t[:, wi * 128 : wi * 128 + 128]
                            sw = s_t[:, wi * 128 : wi * 128 + 128]
                            cexp_ps = pops.tile([128, 128], F32, tag="pcexp")
                            nc.tensor.matmul(cexp_ps[:], E4[:], cw, start=True, stop=True)
                            sexp_ps = pops.tile([128, 128], F32, tag="psexp")
                            nc.tensor.matmul(sexp_ps[:], E4si[:], sw, start=True, stop=True)
                            sexp = post.tile([128, 128], F32, tag="psexp_sb")
                            nc.vector.tensor_copy(sexp[:], sexp_ps[:])
                            xsw_ps = pops.tile([128, 128], F32, tag="pxsw")
                            nc.tensor.matmul(xsw_ps[:], P128[:], acc[:], start=True, stop=True)
                            t1 = post.tile([128, 128], F32, tag="pt1")
                            nc.vector.tensor_tensor(
                                t1[:], cexp_ps[:], acc[:], mybir.AluOpType.mult
                            )
                            t2 = post.tile([128, 128], F32, tag="pt2")
                            nc.vector.tensor_tensor(
                                t2[:], xsw_ps[:], sexp[:], mybir.AluOpType.mult
                            )
                            o1 = post.tile([128, 128], F32, tag="po1")
                            nc.vector.tensor_tensor(
                                o1[:], t1[:], t2[:], mybir.AluOpType.add
                            )
                            o2 = post.tile([128, 128], F32, tag="po2")
                            nc.vector.tensor_tensor(
                                o2[:],
                                o1[:],
                                xt[:, wi * 128 : wi * 128 + 128],
                                mybir.AluOpType.add,
                            )
                            nc.sync.dma_start(
                                out=out_d[:, w * 128 : (w + 1) * 128], in_=o2[:]
                            )

    nc.compile()
    return nc


# ---------------------------------------------------------------- runner
_CACHE = {}


def _constants():
    iota = np.tile(np.arange(128, dtype=np.float32), (128, 1))
    E4 = np.zeros((32, 128), np.float32)
    E4s = np.zeros((32, 128), np.float32)
    for b in range(32):
        for j in range(4):
            E4[b, 4 * b + j] = 1.0
            E4s[b, 4 * b + j] = -1.0 if j < 2 else 1.0
    P = np.zeros((128, 128), np.float32)
    for p in range(128):
        P[p, p ^ 2] = 1.0
    ID = np.eye(128, dtype=np.float32)
    return iota, E4, E4s, -E4s, P, ID


def kernel(
    x,
    edge_index,
    lin_w,
    lin_b,
    enc_w1,
    enc_b1,
    enc_w2,
    enc_b2,
    attention,
    _debug_out=None,
):
    x = np.asarray(x, np.float32)
    key = (hash(np.asarray(edge_index)[:, ::997].tobytes()), _debug_out)
    if key not in _CACHE:
        prep = _preprocess(np.asarray(edge_index))
        nc = _build(prep["sched"], prep["totcols"], debug_out=_debug_out)
        _CACHE[key] = (prep, nc)
    prep, nc = _CACHE[key]

    iota, E4, E4s, E4si, P, ID = _constants()
    deg = prep["deg"]
    in_maps = []
    for c in range(NCORES):
        xs = np.zeros((SLOTS, 128), np.float32)
        xs[:SHARD] = x[c * SHARD : (c + 1) * SHARD]
        degs = np.ones(SLOTS, np.float32)
        degs[:SHARD] = deg[c * SHARD : (c + 1) * SHARD]
        in_maps.append(
            {
                "xT": np.ascontiguousarray(xs.T),
                "idxs": prep["idx_packs"][c],
                "dstloc": prep["dst_packs"][c],
                "degc": np.ascontiguousarray(degs.reshape(WIN, 128).T),
                "encw1": np.asarray(enc_w1, np.float32),
                "encb1": np.asarray(enc_b1, np.float32).reshape(64, 1),
                "encw2": np.asarray(enc_w2, np.float32),
                "encb2": np.asarray(enc_b2, np.float32).reshape(32, 1),
                "linw": np.asarray(lin_w, np.float32),
                "linb": np.asarray(lin_b, np.float32).reshape(128, 1),
                "attT": np.ascontiguousarray(np.asarray(attention, np.float32).T),
                "iotar": iota,
                "E4": E4,
                "E4s": E4s,
                "E4si": E4si,
                "P128": P,
                "ID128": ID,
            }
        )

    res = run_bass_kernel_spmd(nc, in_maps, core_ids=list(range(NCORES)))
    out = np.empty((N, 128), np.float32)
    for c in range(NCORES):
        oc = res.results[c]["out"]  # [128, SLOTS] feature-major
        if isinstance(_debug_out, int) and _debug_out is not None and not isinstance(_debug_out, bool):
            h = oc.reshape(128, WIN, 128).transpose(1, 0, 2).reshape(SLOTS, 128)
            out[c * SHARD : (c + 1) * SHARD] = h[:SHARD]
        else:
            out[c * SHARD : (c + 1) * SHARD] = oc[:, :SHARD].T
    return out
